# revision 25
# baseline (speedup 1.0000x reference)
"""DigitCapsules Trainium2 kernel (8-core batch-data-parallel).

Math (per sample b):
  caps[k,o,hw] = sum_c x[c,hw] conv_w[k,o,c] + conv_b[k,o]
  prim[k,p,d]  = caps[k, g//36, g%36],  g = p*8+d    (pure reindex)
  u[k,p,e]     = sum_d prim[k,p,d] W2[k,p,d,e]
  3 rounds of routing-by-agreement -> v[k,e]

Device mapping per core (256 samples = 2 tiles of 128 on partitions):
  * conv: 36x2 bf16 matmuls, stationary xT[c;b] per hw (contiguous, FWL),
    rhs conv_w[c;ko]; evac to caps[b; k*576+g] bf16
  * caps -> primT[(k,p,d); b] via 45 xbar DMA transposes (128-col chunks;
    chunk t == pair-group t because 576 = 72*8)
  * u-step: 45 block-diagonal matmuls primT_t @ W2blk_t -> psum; evacuated
    twice: u1[b; k,p,e] and u2[b; k,e,p] so both routing multiplies run in
    the DVE 2x bf16 mode. An extra cheap rhs (W2 summed over p, x0.1)
    accumulates s1 = iteration-1 weighted sum for free on the PE.
  * routing on DVE/ACT: softmax over k, weighted sums as 2x TT-mult +
    binary-tree reduction, squash with NR-refined sqrt.
"""

import sys

sys.path.insert(0, "/opt/trn_rl_repo")

import numpy as np
import ml_dtypes

import concourse.bass as bass
import concourse.mybir as mybir
from concourse import tile
from concourse.vector_clock import ScopedClock
from concourse.masks import make_identity
from concourse.bass_utils import run_bass_kernel_spmd

# ---------------------------------------------------------------- constants
K, O, C, H, W = 10, 16, 256, 6, 6
HW = H * W                      # 36
PD, E, P = 8, 16, 72            # prim dim, out dim, prims per k
PAIRS = K * P                   # 720
GROUPS = PAIRS // 16            # 45 groups of 16 (k,p) pairs
KPE = PAIRS * E                 # 11520
KE = K * E                      # 160
KO = K * O                      # 160
B_TOTAL, N_CORES = 2048, 8
B_CORE = B_TOTAL // N_CORES     # 256
TB = 128                        # batch tile (partitions)
NT = B_CORE // TB               # 2
EPS = 1e-8

F32 = mybir.dt.float32
BF16 = mybir.dt.bfloat16


# ------------------------------------------------- tile drain-limit patch
# This walrus build accepts at most 1 sync wait on several instruction
# structs (CTRL drain, S3_LW ldweights, DMA pseudo). Tile piles one wait per
# outstanding sem lane onto single instructions; spill the extras onto
# same-engine NoOps placed immediately before (waits move earlier ->
# strictly more conservative, still correct).
def _patched_drain_and_barrier(self, tick_clock, wait_clock):
    nc = self.nc
    drain_inst = nc.sync.drain()
    wait_clock.add_sem_waits(
        drain_inst.ins, ScopedClock({None: tick_clock.global_clock})
    )
    ow = list(drain_inst.ins.sync_info.on_wait)
    if len(ow) > 1:
        drain_inst.ins.sync_info.on_wait = ow[:1]
        for w in ow[1:]:
            ni = nc.sync.nop()
            ni.ins.sync_info = mybir.SyncInfo(on_wait=[w], on_update=[])
    nc.all_engine_barrier()
    assert self.sems is not None
    popped = nc._tile_sem_poison_stack.pop()
    assert popped is self._sem_poison
    nc.clear_and_free_semaphores(list(self.sems.allocated().values()))
    nc.all_engine_barrier()


tile.TileContext._drain_and_barrier = _patched_drain_and_barrier

_MAXW = 1  # max sync waits this walrus accepts per instruction
_wsplit_n = [0]


def _split_excess_waits(nc):
    for f in nc.m.functions:
        for blk in f.blocks:
            insts = list(blk.instructions)
            out = []
            for inst in insts:
                si = inst.sync_info
                if si is not None and si.on_wait and len(si.on_wait) > _MAXW:
                    ow = list(si.on_wait)
                    extra, keep = ow[:-_MAXW], ow[-_MAXW:]
                    for i in range(0, len(extra), _MAXW):
                        _wsplit_n[0] += 1
                        out.append(
                            mybir.InstNoOp(
                                name=f"I-wsplit-{_wsplit_n[0]}",
                                engine=inst.engine,
                                sync_info=mybir.SyncInfo(
                                    on_wait=extra[i:i + _MAXW], on_update=[]
                                ),
                            )
                        )
                    si.on_wait = keep
                out.append(inst)
            if len(out) != len(insts):
                blk.instructions[:] = out


def _squash(nc, rp, s_ap, v_f32, v_bf16):
    """v = (|s|^2/(1+|s|^2)) * s / sqrt(|s|^2+eps); s_ap [128, K*E] f32."""
    t2 = rp.tile([TB, KE], F32, tag="sq_t2")
    nc.vector.tensor_mul(t2[:], s_ap, s_ap)
    ss = rp.tile([TB, K], F32, tag="sq_ss")
    nc.vector.reduce_sum(
        ss[:], t2[:].rearrange("b (k e) -> b k e", k=K), axis=mybir.AxisListType.X
    )
    # rsqrt(ss+eps) = exp(-0.5*ln(ss+eps)) — ln/exp share one ACT table set
    # with the softmax exp, so no table switches.
    ssq = rp.tile([TB, K], F32, tag="sq_ssq")
    nc.vector.tensor_scalar_add(ssq[:], ss[:], EPS)
    lg = rp.tile([TB, K], F32, tag="sq_lg")
    nc.scalar.activation(lg[:], ssq[:], mybir.ActivationFunctionType.Ln)
    rs = rp.tile([TB, K], F32, tag="sq_rs")
    nc.scalar.activation(
        rs[:], lg[:], mybir.ActivationFunctionType.Exp, scale=-0.5
    )
    a1 = rp.tile([TB, K], F32, tag="sq_a1")
    nc.vector.tensor_scalar_add(a1[:], ss[:], 1.0)
    rd = rp.tile([TB, K], F32, tag="sq_rd")
    nc.vector.reciprocal(rd[:], a1[:])
    f = rp.tile([TB, K], F32, tag="sq_f")
    nc.vector.tensor_mul(f[:], ss[:], rd[:])        # ss/(1+ss)
    nc.vector.tensor_mul(f[:], f[:], rs[:])         # * rsqrt(ss+eps)
    fb = f[:, :, None].broadcast_to([TB, K, E])
    nc.vector.tensor_mul(
        v_f32.rearrange("b (k e) -> b k e", k=K),
        s_ap.rearrange("b (k e) -> b k e", k=K),
        fb,
    )
    nc.scalar.copy(v_bf16, v_f32)


def _k_segments(pr0, np_):
    """Split pair range [pr0, pr0+np_) at k boundaries (multiples of P)."""
    segs = []
    a = pr0
    end = pr0 + np_
    while a < end:
        b = min(end, (a // P + 1) * P)
        segs.append((a, b))
        a = b
    return segs


def _build_nc(has_bias):
    nc = bass.Bass()
    x_d = nc.dram_tensor("xt", [NT, 2, 128, TB * HW], BF16, kind="ExternalInput")
    cw_d = nc.dram_tensor("cwt", [C, KO], BF16, kind="ExternalInput")
    w2_d = nc.dram_tensor("w2blk", [GROUPS, 128, 256], BF16, kind="ExternalInput")
    w2s_d = nc.dram_tensor("w2s", [GROUPS, 128, 32], BF16, kind="ExternalInput")
    w2r_d = nc.dram_tensor("w2r", [KE, PAIRS * PD], BF16, kind="ExternalInput")
    if has_bias:
        bu_d = nc.dram_tensor("biasu", [TB, KPE], BF16, kind="ExternalInput")
        bu2_d = nc.dram_tensor("biasu2", [TB, KPE], BF16, kind="ExternalInput")
        bs_d = nc.dram_tensor("biass1", [TB, KE], F32, kind="ExternalInput")
    out_d = nc.dram_tensor("out", [B_CORE, KE], F32, kind="ExternalOutput")

    with tile.TileContext(nc) as tc:
        with (
            tc.tile_pool(name="consts", bufs=1) as consts,
            tc.tile_pool(name="xp", bufs=1) as xp,
            tc.tile_pool(name="big", bufs=1) as big,
            tc.tile_pool(name="up", bufs=2) as up,
            tc.tile_pool(name="rp", bufs=1) as rp,
            tc.tile_pool(name="ps_conv", bufs=2, space="PSUM") as ps_conv,
            tc.tile_pool(name="ps_tr", bufs=1, space="PSUM") as ps_tr,
            tc.tile_pool(name="ps_u", bufs=2, space="PSUM") as ps_u,
            tc.tile_pool(name="ps_s1", bufs=1, space="PSUM") as ps_s1p,
        ):
            cw0 = consts.tile([128, KO], BF16)
            cw1 = consts.tile([128, KO], BF16)
            nc.sync.dma_start(cw0[:], cw_d[0:128, :])
            nc.sync.dma_start(cw1[:], cw_d[128:256, :])
            w2t = consts.tile([128, GROUPS * 256], BF16)
            nc.sync.dma_start(
                w2t[:].rearrange("c (t n) -> c t n", t=GROUPS),
                w2_d[:].rearrange("t c n -> c t n"),
            )
            ident = consts.tile([128, 128], BF16)
            make_identity(nc, ident[:])
            w2r0 = consts.tile([128, PAIRS * PD], BF16)
            nc.sync.dma_start(w2r0[:], w2r_d[0:128, :])
            w2r1 = consts.tile([32, PAIRS * PD], BF16)
            nc.sync.dma_start(w2r1[:], w2r_d[128:KE, :])
            w2st = consts.tile([128, GROUPS * 32], BF16)
            nc.sync.dma_start(
                w2st[:].rearrange("c (t n) -> c t n", t=GROUPS),
                w2s_d[:].rearrange("t c n -> c t n"),
            )
            if has_bias:
                but = consts.tile([TB, KPE], BF16)
                nc.sync.dma_start(but[:], bu_d[:])
                but2 = consts.tile([TB, KPE], BF16)
                nc.sync.dma_start(but2[:], bu2_d[:])
                bst = consts.tile([TB, KE], F32)
                nc.sync.dma_start(bst[:], bs_d[:])

            for bt in range(NT):
                # ------------------------------------------------ x load
                xt0 = xp.tile([128, TB * HW], BF16, tag="xt0")
                xt1 = xp.tile([128, TB * HW], BF16, tag="xt1")
                nc.sync.dma_start(xt0[:], x_d[bt, 0, :, :])
                nc.sync.dma_start(xt1[:], x_d[bt, 1, :, :])

                # ------------------------------------------------ conv
                caps = up.tile([TB, K * O * HW], BF16, tag="caps")
                capsv = caps[:].rearrange("b (k o hw) -> b k o hw", k=K, o=O)
                for hb in range(HW // 3):  # 12 psum banks of 3 hw each
                    pc = ps_conv.tile([TB, 3 * KO], F32, tag="pconv")
                    for j in range(3):
                        hw = hb * 3 + j
                        nc.tensor.matmul(
                            pc[:, j * KO:(j + 1) * KO],
                            xt0[:, hw * TB:(hw + 1) * TB],
                            cw0[:],
                            start=True, stop=False,
                        )
                        nc.tensor.matmul(
                            pc[:, j * KO:(j + 1) * KO],
                            xt1[:, hw * TB:(hw + 1) * TB],
                            cw1[:],
                            start=False, stop=True,
                        )
                    # evac [b; j,k,o] -> caps[b; k,o,hw=hb*3+j]
                    ev = (nc.vector.tensor_copy if (bt == 0 and hb % 2 == 0)
                          else nc.scalar.copy)
                    ev(
                        capsv[:, :, :, hb * 3:hb * 3 + 3],
                        pc[:].rearrange("b (j k o) -> b k o j", j=3, k=K),
                    )

                # ------------------------------- transpose to primT (PE)
                primT = big.tile([128, GROUPS * 128], BF16, tag="primT")
                for tb5 in range(GROUPS // 5):  # 9 psum tiles of 5 chunks
                    pt = ps_tr.tile([128, 5 * 128], BF16, tag="ptr")
                    for j in range(5):
                        t = tb5 * 5 + j
                        nc.tensor.transpose(
                            pt[:, j * 128:(j + 1) * 128],
                            caps[:, t * 128:(t + 1) * 128],
                            ident[:],
                        )
                    eng = (nc.vector.tensor_copy if (bt == 0 and tb5 % 2 == 0)
                           else nc.scalar.copy)
                    eng(primT[:, tb5 * 640:(tb5 + 1) * 640], pt[:])

                # ------------------------------------------------ u-step
                u2 = up.tile([TB, KPE], BF16, tag="u2")
                u2v = u2[:].rearrange("b (k e p) -> b k e p", k=K, e=E)
                if has_bias:
                    u1 = up.tile([TB, KPE], BF16, tag="u1")
                    u1v = u1[:].rearrange("b (k p e) -> b k p e", k=K, p=P)
                ps1 = ps_s1p.tile([TB, 176], F32, tag="ps1")
                for ub in range(12):  # psum tiles of <=4 groups (2 banks)
                    ng = min(4, GROUPS - 4 * ub)
                    pu = ps_u.tile([TB, 4 * 256], F32, tag="pu")
                    for j in range(ng):
                        t = ub * 4 + j
                        nc.tensor.matmul(
                            pu[:, j * 256:(j + 1) * 256],
                            primT[:, t * 128:(t + 1) * 128],
                            w2t[:, t * 256:(t + 1) * 256],
                            start=True, stop=True,
                        )
                        k0 = (16 * t) // P
                        nc.tensor.matmul(
                            ps1[:, k0 * 16:k0 * 16 + 32],
                            primT[:, t * 128:(t + 1) * 128],
                            w2st[:, t * 32:(t + 1) * 32],
                            start=(t == 0), stop=(t == GROUPS - 1),
                            skip_group_check=True,
                        )
                    if has_bias:
                        nc.scalar.copy(
                            u1[:, ub * 1024:ub * 1024 + ng * 256], pu[:, :ng * 256]
                        )
                    # u2 evac [b; k,e,p], split at k boundaries
                    pr0 = ub * 64
                    puv = pu[:].rearrange("b (pr e) -> b pr e", e=E)
                    for (a, b2) in _k_segments(pr0, ng * 16):
                        k = a // P
                        nc.scalar.copy(
                            u2v[:, k, :, a - k * P:b2 - k * P],
                            puv[:, a - pr0:b2 - pr0, :].rearrange("b pr e -> b e pr"),
                        )
                if has_bias:
                    nc.vector.tensor_add(u1[:], u1[:], but[:])
                    nc.vector.tensor_add(u2[:], u2[:], but2[:])
                w2v = big.tile([TB, PAIRS * PD], BF16, tag="w2v")

                # ------------------------------------------------ routing
                s_sb = rp.tile([TB, KE], F32, tag="s_sb")
                nc.scalar.copy(s_sb[:], ps1[:, :KE])
                if has_bias:
                    nc.vector.tensor_add(s_sb[:], s_sb[:], bst[:])
                v32 = rp.tile([TB, KE], F32, tag="v32")
                vbf = rp.tile([TB, KE], BF16, tag="vbf")
                _squash(nc, rp, s_sb[:], v32[:], vbf[:])

                bl = rp.tile([TB, PAIRS], F32, tag="bl")
                tmp = big.tile([TB, KPE], BF16, tag="tmp")
                tmpv = tmp[:].rearrange("b (k p e) -> b k p e", k=K, p=P)
                trh = big.tile([TB, KPE // 2], BF16, tag="trh")
                trhe = trh[:].rearrange("b (kp e) -> b kp e", e=8)

                for it in range(3):
                    if it > 0:
                        # softmax over k -> c, then s = sum_p c*u  (u2 layout)
                        eb = rp.tile([TB, PAIRS], BF16, tag="eb")
                        nc.scalar.activation(
                            eb[:], bl[:], mybir.ActivationFunctionType.Exp
                        )
                        z = rp.tile([TB, P], F32, tag="z")
                        nc.vector.reduce_sum(
                            z[:],
                            eb[:].rearrange("b (k p) -> b p k", k=K),
                            axis=mybir.AxisListType.X,
                        )
                        rz = rp.tile([TB, P], F32, tag="rz")
                        nc.vector.reciprocal(rz[:], z[:])
                        rzb = rp.tile([TB, P], BF16, tag="rzb")
                        nc.scalar.copy(rzb[:], rz[:])
                        cbf = rp.tile([TB, PAIRS], BF16, tag="cbf")
                        nc.vector.tensor_mul(
                            cbf[:].rearrange("b (k p) -> b k p", k=K),
                            eb[:].rearrange("b (k p) -> b k p", k=K),
                            rzb[:, None, :].broadcast_to([TB, K, P]),
                        )
                        # tmp[b; (k,e), p] = u2 * c  (2x bf16)
                        t2v = tmp[:].rearrange("b (ke p) -> b ke p", p=P)
                        trhp = trh[:].rearrange("b (ke p) -> b ke p", p=36)
                        nc.vector.tensor_mul(
                            tmp[:].rearrange("b (k e p) -> b k e p", k=K, e=E),
                            u2v,
                            cbf[:].rearrange("b (k p) -> b k p", k=K)[:, :, None, :]
                            .broadcast_to([TB, K, E, P]),
                        )
                        # tree over p: 72->36->18->9 then X-reduce
                        nc.vector.tensor_add(
                            trhp[:, :, :], t2v[:, :, 0:36], t2v[:, :, 36:72]
                        )
                        nc.vector.tensor_add(
                            t2v[:, :, 0:18], trhp[:, :, 0:18], trhp[:, :, 18:36]
                        )
                        nc.vector.tensor_add(
                            trhp[:, :, 0:9], t2v[:, :, 0:9], t2v[:, :, 9:18]
                        )
                        nc.vector.reduce_sum(
                            s_sb[:].rearrange("b (k e) -> b k e", k=K),
                            trhp[:, :, 0:9],
                            axis=mybir.AxisListType.X,
                        )
                        _squash(nc, rp, s_sb[:], v32[:], vbf[:])
                    if it < 2:
                        if not has_bias:
                            # agreement via PE: W2v[b,(k,p,d)] = sum_e v*W2,
                            # then bl += sum_d caps (.) W2v
                            pv0 = ps_tr.tile([128, 128], BF16, tag="ptr")
                            nc.tensor.transpose(pv0[:], vbf[:, 0:128], ident[:])
                            vt0 = rp.tile([128, 128], BF16, tag="vt0")
                            nc.scalar.copy(vt0[:], pv0[:])
                            pv1 = ps_tr.tile([32, 128], BF16, tag="ptr")
                            nc.tensor.transpose(pv1[:], vbf[:, 128:KE], ident[:])
                            vt1 = rp.tile([32, 128], BF16, tag="vt1")
                            nc.scalar.copy(vt1[:], pv1[:])
                            for q in range(12):
                                off = q * 480
                                pw = ps_conv.tile([TB, 480], F32, tag="pconv")
                                nc.tensor.matmul(
                                    pw[:], vt0[:], w2r0[:, off:off + 480],
                                    start=True, stop=False,
                                )
                                nc.tensor.matmul(
                                    pw[:], vt1[:], w2r1[:, off:off + 480],
                                    start=False, stop=True,
                                )
                                nc.scalar.copy(w2v[:, off:off + 480], pw[:])
                            nc.vector.tensor_mul(trh[:], caps[:], w2v[:])
                            trhd = trh[:].rearrange("b (kp d) -> b kp d", d=PD)
                            tv4 = tmp[:].rearrange("b (kp d) -> b kp d", d=4)
                            nc.vector.tensor_add(
                                tv4[:, 0:PAIRS, :], trhd[:, :, 0:4], trhd[:, :, 4:8]
                            )
                            tr2 = trh[:, 0:PAIRS * 2].rearrange(
                                "b (kp d) -> b kp d", d=2
                            )
                            nc.vector.tensor_add(
                                tr2, tv4[:, 0:PAIRS, 0:2], tv4[:, 0:PAIRS, 2:4]
                            )
                            if it == 0:
                                nc.vector.tensor_add(
                                    bl[:], tr2[:, :, 0], tr2[:, :, 1]
                                )
                            else:
                                bld = rp.tile([TB, PAIRS], F32, tag="bld")
                                nc.vector.tensor_add(
                                    bld[:], tr2[:, :, 0], tr2[:, :, 1]
                                )
                                nc.vector.tensor_add(bl[:], bl[:], bld[:])
                        else:
                            # general path: bl += sum_e u1*v on DVE
                            nc.vector.tensor_mul(
                                tmpv,
                                u1v,
                                vbf[:].rearrange("b (k e) -> b k e", k=K)[:, :, None, :]
                                .broadcast_to([TB, K, P, E]),
                            )
                            tv = tmp[:].rearrange("b (kp e) -> b kp e", e=E)
                            nc.vector.tensor_add(
                                trhe[:, :, :], tv[:, :, 0:8], tv[:, :, 8:16]
                            )
                            nc.vector.tensor_add(
                                tv[:, :, 0:4], trhe[:, :, 0:4], trhe[:, :, 4:8]
                            )
                            nc.vector.tensor_add(
                                trhe[:, :, 0:2], tv[:, :, 0:2], tv[:, :, 2:4]
                            )
                            if it == 0:
                                nc.vector.tensor_add(
                                    bl[:], trhe[:, :, 0], trhe[:, :, 1]
                                )
                            else:
                                bld = rp.tile([TB, PAIRS], F32, tag="bld")
                                nc.vector.tensor_add(
                                    bld[:], trhe[:, :, 0], trhe[:, :, 1]
                                )
                                nc.vector.tensor_add(bl[:], bl[:], bld[:])

                nc.sync.dma_start(out_d[bt * TB:(bt + 1) * TB, :], v32[:])
    _split_excess_waits(nc)
    return nc


_NC_CACHE = {}


def kernel(x, conv_w, conv_b, weights, _trace=False):
    x = np.asarray(x, dtype=np.float32)
    conv_w = np.asarray(conv_w, dtype=np.float32)
    conv_b = np.asarray(conv_b, dtype=np.float32)
    weights = np.asarray(weights, dtype=np.float32)

    # ---------------- host-side weight packing (tiny, O(weights))
    cwT = conv_w.transpose(2, 0, 1).reshape(C, KO).astype(ml_dtypes.bfloat16)
    w2blk = np.zeros((GROUPS, 128, 256), np.float32)
    w2s = np.zeros((GROUPS, 128, 32), np.float32)
    for t in range(GROUPS):
        k0 = (16 * t) // P
        for i in range(16):
            k, p = divmod(16 * t + i, P)
            w2blk[t, i * 8:(i + 1) * 8, i * 16:(i + 1) * 16] = weights[k, p]
            w2s[t, i * 8:(i + 1) * 8, (k - k0) * 16:(k - k0 + 1) * 16] += (
                0.1 * weights[k, p]
            )
    w2blk = w2blk.astype(ml_dtypes.bfloat16)
    w2r = np.zeros((KE, PAIRS * PD), np.float32)
    for k in range(K):
        w2r[k * 16:(k + 1) * 16, k * 576:(k + 1) * 576] = (
            weights[k].transpose(2, 0, 1).reshape(E, P * PD)
        )
    w2r = w2r.astype(ml_dtypes.bfloat16)
    w2s = w2s.astype(ml_dtypes.bfloat16)

    has_bias = bool(np.any(conv_b))
    extra = {}
    if has_bias:
        g = np.arange(P * PD)
        o_of = (g // HW).reshape(P, PD)
        bU = np.einsum("kpd,kpde->kpe", conv_b[:, o_of], weights)
        bs1 = 0.1 * bU.sum(1)
        extra["biasu"] = np.broadcast_to(
            bU.reshape(1, KPE).astype(ml_dtypes.bfloat16), (TB, KPE)
        ).copy()
        extra["biasu2"] = np.broadcast_to(
            bU.transpose(0, 2, 1).reshape(1, KPE).astype(ml_dtypes.bfloat16),
            (TB, KPE),
        ).copy()
        extra["biass1"] = np.broadcast_to(
            bs1.reshape(1, KE).astype(np.float32), (TB, KE)
        ).copy()

    # -------- shard + transpose x on host: [core][bt, chunk, c, hw, b]
    xb = x.reshape(B_TOTAL, C, HW).astype(ml_dtypes.bfloat16)
    in_maps = []
    for ci in range(N_CORES):
        xs = xb[ci * B_CORE:(ci + 1) * B_CORE]            # [256, 256, 36]
        xs = xs.reshape(NT, TB, 2, 128, HW)               # bt, b, chunk, c, hw
        xT = np.ascontiguousarray(xs.transpose(0, 2, 3, 4, 1))  # bt,chunk,c,hw,b
        in_maps.append(
            {
                "xt": xT.reshape(NT, 2, 128, TB * HW),
                "cwt": cwT,
                "w2blk": w2blk,
                "w2s": w2s,
                "w2r": w2r,
                **extra,
            }
        )

    key = has_bias
    if key not in _NC_CACHE:
        _NC_CACHE[key] = _build_nc(has_bias)
    nc = _NC_CACHE[key]

    res = run_bass_kernel_spmd(
        nc, in_maps, core_ids=list(range(N_CORES)), trace=_trace
    )
    out = np.concatenate([r["out"] for r in res.results], axis=0)
    if _trace:
        kernel._last_result = res
    return out.reshape(B_TOTAL, K, E)


# revision 26
# speedup vs baseline: 1.1210x; 1.1210x over previous
"""DigitCapsules Trainium2 kernel (8-core batch-data-parallel).

Math (per sample b):
  caps[k,o,hw] = sum_c x[c,hw] conv_w[k,o,c] + conv_b[k,o]
  prim[k,p,d]  = caps[k, g//36, g%36],  g = p*8+d    (pure reindex)
  u[k,p,e]     = sum_d prim[k,p,d] W2[k,p,d,e]
  3 rounds of routing-by-agreement -> v[k,e]

Device mapping per core (256 samples = 2 tiles of 128 on partitions):
  * conv: 36x2 bf16 matmuls, stationary xT[c;b] per hw (contiguous, FWL),
    rhs conv_w[c;ko]; evac to caps[b; k*576+g] bf16
  * caps -> primT[(k,p,d); b] via 45 xbar DMA transposes (128-col chunks;
    chunk t == pair-group t because 576 = 72*8)
  * u-step: 45 block-diagonal matmuls primT_t @ W2blk_t -> psum; evacuated
    twice: u1[b; k,p,e] and u2[b; k,e,p] so both routing multiplies run in
    the DVE 2x bf16 mode. An extra cheap rhs (W2 summed over p, x0.1)
    accumulates s1 = iteration-1 weighted sum for free on the PE.
  * routing on DVE/ACT: softmax over k, weighted sums as 2x TT-mult +
    binary-tree reduction, squash with NR-refined sqrt.
"""

import sys

sys.path.insert(0, "/opt/trn_rl_repo")

import numpy as np
import ml_dtypes

import concourse.bass as bass
import concourse.mybir as mybir
from concourse import tile
from concourse.vector_clock import ScopedClock
from concourse.masks import make_identity
from concourse.bass_utils import run_bass_kernel_spmd

# ---------------------------------------------------------------- constants
K, O, C, H, W = 10, 16, 256, 6, 6
HW = H * W                      # 36
PD, E, P = 8, 16, 72            # prim dim, out dim, prims per k
PAIRS = K * P                   # 720
GROUPS = PAIRS // 16            # 45 groups of 16 (k,p) pairs
KPE = PAIRS * E                 # 11520
KE = K * E                      # 160
KO = K * O                      # 160
B_TOTAL, N_CORES = 2048, 8
B_CORE = B_TOTAL // N_CORES     # 256
TB = 128                        # batch tile (partitions)
NT = B_CORE // TB               # 2
EPS = 1e-8

F32 = mybir.dt.float32
BF16 = mybir.dt.bfloat16


# ------------------------------------------------- tile drain-limit patch
# This walrus build accepts at most 1 sync wait on several instruction
# structs (CTRL drain, S3_LW ldweights, DMA pseudo). Tile piles one wait per
# outstanding sem lane onto single instructions; spill the extras onto
# same-engine NoOps placed immediately before (waits move earlier ->
# strictly more conservative, still correct).
def _patched_drain_and_barrier(self, tick_clock, wait_clock):
    nc = self.nc
    drain_inst = nc.sync.drain()
    wait_clock.add_sem_waits(
        drain_inst.ins, ScopedClock({None: tick_clock.global_clock})
    )
    ow = list(drain_inst.ins.sync_info.on_wait)
    if len(ow) > 1:
        drain_inst.ins.sync_info.on_wait = ow[:1]
        for w in ow[1:]:
            ni = nc.sync.nop()
            ni.ins.sync_info = mybir.SyncInfo(on_wait=[w], on_update=[])
    nc.all_engine_barrier()
    assert self.sems is not None
    popped = nc._tile_sem_poison_stack.pop()
    assert popped is self._sem_poison
    nc.clear_and_free_semaphores(list(self.sems.allocated().values()))
    nc.all_engine_barrier()


tile.TileContext._drain_and_barrier = _patched_drain_and_barrier

_MAXW = 1  # max sync waits this walrus accepts per instruction
_wsplit_n = [0]


def _split_excess_waits(nc):
    for f in nc.m.functions:
        for blk in f.blocks:
            insts = list(blk.instructions)
            out = []
            for inst in insts:
                si = inst.sync_info
                if si is not None and si.on_wait and len(si.on_wait) > _MAXW:
                    ow = list(si.on_wait)
                    extra, keep = ow[:-_MAXW], ow[-_MAXW:]
                    for i in range(0, len(extra), _MAXW):
                        _wsplit_n[0] += 1
                        out.append(
                            mybir.InstNoOp(
                                name=f"I-wsplit-{_wsplit_n[0]}",
                                engine=inst.engine,
                                sync_info=mybir.SyncInfo(
                                    on_wait=extra[i:i + _MAXW], on_update=[]
                                ),
                            )
                        )
                    si.on_wait = keep
                out.append(inst)
            if len(out) != len(insts):
                blk.instructions[:] = out


def _squash(nc, rp, s_ap, v_f32, v_bf16):
    """v = (|s|^2/(1+|s|^2)) * s / sqrt(|s|^2+eps); s_ap [128, K*E] f32."""
    t2 = rp.tile([TB, KE], F32, tag="sq_t2")
    nc.vector.tensor_mul(t2[:], s_ap, s_ap)
    ss = rp.tile([TB, K], F32, tag="sq_ss")
    nc.vector.reduce_sum(
        ss[:], t2[:].rearrange("b (k e) -> b k e", k=K), axis=mybir.AxisListType.X
    )
    # rsqrt(ss+eps) = exp(-0.5*ln(ss+eps)) — ln/exp share one ACT table set
    # with the softmax exp, so no table switches.
    ssq = rp.tile([TB, K], F32, tag="sq_ssq")
    nc.vector.tensor_scalar_add(ssq[:], ss[:], EPS)
    lg = rp.tile([TB, K], F32, tag="sq_lg")
    nc.scalar.activation(lg[:], ssq[:], mybir.ActivationFunctionType.Ln)
    rs = rp.tile([TB, K], F32, tag="sq_rs")
    nc.scalar.activation(
        rs[:], lg[:], mybir.ActivationFunctionType.Exp, scale=-0.5
    )
    a1 = rp.tile([TB, K], F32, tag="sq_a1")
    nc.vector.tensor_scalar_add(a1[:], ss[:], 1.0)
    rd = rp.tile([TB, K], F32, tag="sq_rd")
    nc.vector.reciprocal(rd[:], a1[:])
    f = rp.tile([TB, K], F32, tag="sq_f")
    nc.vector.tensor_mul(f[:], ss[:], rd[:])        # ss/(1+ss)
    nc.vector.tensor_mul(f[:], f[:], rs[:])         # * rsqrt(ss+eps)
    fb = f[:, :, None].broadcast_to([TB, K, E])
    nc.vector.tensor_mul(
        v_f32.rearrange("b (k e) -> b k e", k=K),
        s_ap.rearrange("b (k e) -> b k e", k=K),
        fb,
    )
    nc.scalar.copy(v_bf16, v_f32)


def _k_segments(pr0, np_):
    """Split pair range [pr0, pr0+np_) at k boundaries (multiples of P)."""
    segs = []
    a = pr0
    end = pr0 + np_
    while a < end:
        b = min(end, (a // P + 1) * P)
        segs.append((a, b))
        a = b
    return segs


def _build_nc(has_bias):
    nc = bass.Bass()
    x_d = nc.dram_tensor("xt", [NT, 2, 128, TB * HW], BF16, kind="ExternalInput")
    cw_d = nc.dram_tensor("cwt", [C, KO], BF16, kind="ExternalInput")
    w2_d = nc.dram_tensor("w2blk", [GROUPS, 128, 256], BF16, kind="ExternalInput")
    w2s_d = nc.dram_tensor("w2s", [GROUPS, 128, 32], BF16, kind="ExternalInput")
    if has_bias:
        bu_d = nc.dram_tensor("biasu", [TB, KPE], BF16, kind="ExternalInput")
        bu2_d = nc.dram_tensor("biasu2", [TB, KPE], BF16, kind="ExternalInput")
        bs_d = nc.dram_tensor("biass1", [TB, KE], F32, kind="ExternalInput")
    out_d = nc.dram_tensor("out", [B_CORE, KE], F32, kind="ExternalOutput")

    with tile.TileContext(nc) as tc:
        with (
            tc.tile_pool(name="consts", bufs=1) as consts,
            tc.tile_pool(name="xp", bufs=1) as xp,
            tc.tile_pool(name="big", bufs=1) as big,
            tc.tile_pool(name="up", bufs=2) as up,
            tc.tile_pool(name="rp", bufs=1) as rp,
            tc.tile_pool(name="ps_conv", bufs=2, space="PSUM") as ps_conv,
            tc.tile_pool(name="ps_tr", bufs=1, space="PSUM") as ps_tr,
            tc.tile_pool(name="ps_u", bufs=2, space="PSUM") as ps_u,
            tc.tile_pool(name="ps_s1", bufs=1, space="PSUM") as ps_s1p,
        ):
            cw0 = consts.tile([128, KO], BF16)
            cw1 = consts.tile([128, KO], BF16)
            nc.sync.dma_start(cw0[:], cw_d[0:128, :])
            nc.sync.dma_start(cw1[:], cw_d[128:256, :])
            w2t = consts.tile([128, GROUPS * 256], BF16)
            nc.sync.dma_start(
                w2t[:].rearrange("c (t n) -> c t n", t=GROUPS),
                w2_d[:].rearrange("t c n -> c t n"),
            )
            ident = consts.tile([128, 128], BF16)
            make_identity(nc, ident[:])
            w2st = consts.tile([128, GROUPS * 32], BF16)
            nc.sync.dma_start(
                w2st[:].rearrange("c (t n) -> c t n", t=GROUPS),
                w2s_d[:].rearrange("t c n -> c t n"),
            )
            if has_bias:
                but = consts.tile([TB, KPE], BF16)
                nc.sync.dma_start(but[:], bu_d[:])
                but2 = consts.tile([TB, KPE], BF16)
                nc.sync.dma_start(but2[:], bu2_d[:])
                bst = consts.tile([TB, KE], F32)
                nc.sync.dma_start(bst[:], bs_d[:])

            for bt in range(NT):
                # ------------------------------------------------ x load
                xt0 = xp.tile([128, TB * HW], BF16, tag="xt0")
                xt1 = xp.tile([128, TB * HW], BF16, tag="xt1")
                nc.sync.dma_start(xt0[:], x_d[bt, 0, :, :])
                nc.sync.dma_start(xt1[:], x_d[bt, 1, :, :])

                # ------------------------------------------------ conv
                caps = big.tile([TB, K * O * HW], BF16, tag="caps")
                capsv = caps[:].rearrange("b (k o hw) -> b k o hw", k=K, o=O)
                for hb in range(HW // 3):  # 12 psum banks of 3 hw each
                    pc = ps_conv.tile([TB, 3 * KO], F32, tag="pconv")
                    for j in range(3):
                        hw = hb * 3 + j
                        nc.tensor.matmul(
                            pc[:, j * KO:(j + 1) * KO],
                            xt0[:, hw * TB:(hw + 1) * TB],
                            cw0[:],
                            start=True, stop=False,
                        )
                        nc.tensor.matmul(
                            pc[:, j * KO:(j + 1) * KO],
                            xt1[:, hw * TB:(hw + 1) * TB],
                            cw1[:],
                            start=False, stop=True,
                        )
                    # evac [b; j,k,o] -> caps[b; k,o,hw=hb*3+j]
                    ev = (nc.vector.tensor_copy if (bt == 0 and hb % 2 == 0)
                          else nc.scalar.copy)
                    ev(
                        capsv[:, :, :, hb * 3:hb * 3 + 3],
                        pc[:].rearrange("b (j k o) -> b k o j", j=3, k=K),
                    )

                # ------------------------------- transpose to primT (PE)
                primT = big.tile([128, GROUPS * 128], BF16, tag="primT")
                for tb5 in range(GROUPS // 5):  # 9 psum tiles of 5 chunks
                    pt = ps_tr.tile([128, 5 * 128], BF16, tag="ptr")
                    for j in range(5):
                        t = tb5 * 5 + j
                        nc.tensor.transpose(
                            pt[:, j * 128:(j + 1) * 128],
                            caps[:, t * 128:(t + 1) * 128],
                            ident[:],
                        )
                    eng = (nc.vector.tensor_copy if (bt == 0 and tb5 % 2 == 0)
                           else nc.scalar.copy)
                    eng(primT[:, tb5 * 640:(tb5 + 1) * 640], pt[:])

                # ------------------------------------------------ u-step
                u2 = up.tile([TB, KPE], BF16, tag="u2")
                u2v = u2[:].rearrange("b (k e p) -> b k e p", k=K, e=E)
                u1 = up.tile([TB, KPE], BF16, tag="u1")
                u1v = u1[:].rearrange("b (k p e) -> b k p e", k=K, p=P)
                ps1 = ps_s1p.tile([TB, 176], F32, tag="ps1")
                for ub in range(12):  # psum tiles of <=4 groups (2 banks)
                    ng = min(4, GROUPS - 4 * ub)
                    pu = ps_u.tile([TB, 4 * 256], F32, tag="pu")
                    for j in range(ng):
                        t = ub * 4 + j
                        nc.tensor.matmul(
                            pu[:, j * 256:(j + 1) * 256],
                            primT[:, t * 128:(t + 1) * 128],
                            w2t[:, t * 256:(t + 1) * 256],
                            start=True, stop=True,
                        )
                        k0 = (16 * t) // P
                        nc.tensor.matmul(
                            ps1[:, k0 * 16:k0 * 16 + 32],
                            primT[:, t * 128:(t + 1) * 128],
                            w2st[:, t * 32:(t + 1) * 32],
                            start=(t == 0), stop=(t == GROUPS - 1),
                            skip_group_check=True,
                        )
                    ev1 = (nc.vector.tensor_copy if (bt == 0 and ub % 2 == 0)
                           else nc.scalar.copy)
                    ev1(u1[:, ub * 1024:ub * 1024 + ng * 256], pu[:, :ng * 256])
                    # u2 evac [b; k,e,p], split at k boundaries
                    pr0 = ub * 64
                    puv = pu[:].rearrange("b (pr e) -> b pr e", e=E)
                    for (a, b2) in _k_segments(pr0, ng * 16):
                        k = a // P
                        nc.scalar.copy(
                            u2v[:, k, :, a - k * P:b2 - k * P],
                            puv[:, a - pr0:b2 - pr0, :].rearrange("b pr e -> b e pr"),
                        )
                if has_bias:
                    nc.vector.tensor_add(u1[:], u1[:], but[:])
                    nc.vector.tensor_add(u2[:], u2[:], but2[:])

                # ------------------------------------------------ routing
                s_sb = rp.tile([TB, KE], F32, tag="s_sb")
                nc.scalar.copy(s_sb[:], ps1[:, :KE])
                if has_bias:
                    nc.vector.tensor_add(s_sb[:], s_sb[:], bst[:])
                v32 = rp.tile([TB, KE], F32, tag="v32")
                vbf = rp.tile([TB, KE], BF16, tag="vbf")
                _squash(nc, rp, s_sb[:], v32[:], vbf[:])

                bl = rp.tile([TB, PAIRS], F32, tag="bl")
                tmp = big.tile([TB, KPE], BF16, tag="tmp")
                tmpv = tmp[:].rearrange("b (k p e) -> b k p e", k=K, p=P)
                trh = big.tile([TB, KPE // 2], BF16, tag="trh")
                trhe = trh[:].rearrange("b (kp e) -> b kp e", e=8)

                for it in range(3):
                    if it > 0:
                        # softmax over k -> c, then s = sum_p c*u  (u2 layout)
                        eb = rp.tile([TB, PAIRS], BF16, tag="eb")
                        nc.scalar.activation(
                            eb[:], bl[:], mybir.ActivationFunctionType.Exp
                        )
                        z = rp.tile([TB, P], F32, tag="z")
                        nc.vector.reduce_sum(
                            z[:],
                            eb[:].rearrange("b (k p) -> b p k", k=K),
                            axis=mybir.AxisListType.X,
                        )
                        rz = rp.tile([TB, P], F32, tag="rz")
                        nc.vector.reciprocal(rz[:], z[:])
                        rzb = rp.tile([TB, P], BF16, tag="rzb")
                        nc.scalar.copy(rzb[:], rz[:])
                        cbf = rp.tile([TB, PAIRS], BF16, tag="cbf")
                        nc.vector.tensor_mul(
                            cbf[:].rearrange("b (k p) -> b k p", k=K),
                            eb[:].rearrange("b (k p) -> b k p", k=K),
                            rzb[:, None, :].broadcast_to([TB, K, P]),
                        )
                        # tmp[b; (k,e), p] = u2 * c  (2x bf16)
                        t2v = tmp[:].rearrange("b (ke p) -> b ke p", p=P)
                        trhp = trh[:].rearrange("b (ke p) -> b ke p", p=36)
                        nc.vector.tensor_mul(
                            tmp[:].rearrange("b (k e p) -> b k e p", k=K, e=E),
                            u2v,
                            cbf[:].rearrange("b (k p) -> b k p", k=K)[:, :, None, :]
                            .broadcast_to([TB, K, E, P]),
                        )
                        # tree over p: 72->36->18->9 then X-reduce
                        nc.vector.tensor_add(
                            trhp[:, :, :], t2v[:, :, 0:36], t2v[:, :, 36:72]
                        )
                        nc.vector.tensor_add(
                            t2v[:, :, 0:18], trhp[:, :, 0:18], trhp[:, :, 18:36]
                        )
                        nc.vector.tensor_add(
                            trhp[:, :, 0:9], t2v[:, :, 0:9], t2v[:, :, 9:18]
                        )
                        nc.vector.reduce_sum(
                            s_sb[:].rearrange("b (k e) -> b k e", k=K),
                            trhp[:, :, 0:9],
                            axis=mybir.AxisListType.X,
                        )
                        _squash(nc, rp, s_sb[:], v32[:], vbf[:])
                    if it < 2:
                        # agreement: bl += sum_e u1*v  (2x bf16 + tree)
                        nc.vector.tensor_mul(
                            tmpv,
                            u1v,
                            vbf[:].rearrange("b (k e) -> b k e", k=K)[:, :, None, :]
                            .broadcast_to([TB, K, P, E]),
                        )
                        tv = tmp[:].rearrange("b (kp e) -> b kp e", e=E)
                        nc.vector.tensor_add(
                            trhe[:, :, :], tv[:, :, 0:8], tv[:, :, 8:16]
                        )
                        nc.vector.tensor_add(
                            tv[:, :, 0:4], trhe[:, :, 0:4], trhe[:, :, 4:8]
                        )
                        nc.vector.tensor_add(
                            trhe[:, :, 0:2], tv[:, :, 0:2], tv[:, :, 2:4]
                        )
                        if it == 0:
                            nc.vector.tensor_add(
                                bl[:], trhe[:, :, 0], trhe[:, :, 1]
                            )
                        else:
                            bld = rp.tile([TB, PAIRS], F32, tag="bld")
                            nc.vector.tensor_add(bld[:], trhe[:, :, 0], trhe[:, :, 1])
                            nc.vector.tensor_add(bl[:], bl[:], bld[:])

                nc.sync.dma_start(out_d[bt * TB:(bt + 1) * TB, :], v32[:])
    _split_excess_waits(nc)
    return nc


_NC_CACHE = {}


def kernel(x, conv_w, conv_b, weights, _trace=False):
    x = np.asarray(x, dtype=np.float32)
    conv_w = np.asarray(conv_w, dtype=np.float32)
    conv_b = np.asarray(conv_b, dtype=np.float32)
    weights = np.asarray(weights, dtype=np.float32)

    # ---------------- host-side weight packing (tiny, O(weights))
    cwT = conv_w.transpose(2, 0, 1).reshape(C, KO).astype(ml_dtypes.bfloat16)
    w2blk = np.zeros((GROUPS, 128, 256), np.float32)
    w2s = np.zeros((GROUPS, 128, 32), np.float32)
    for t in range(GROUPS):
        k0 = (16 * t) // P
        for i in range(16):
            k, p = divmod(16 * t + i, P)
            w2blk[t, i * 8:(i + 1) * 8, i * 16:(i + 1) * 16] = weights[k, p]
            w2s[t, i * 8:(i + 1) * 8, (k - k0) * 16:(k - k0 + 1) * 16] += (
                0.1 * weights[k, p]
            )
    w2blk = w2blk.astype(ml_dtypes.bfloat16)
    w2s = w2s.astype(ml_dtypes.bfloat16)

    has_bias = bool(np.any(conv_b))
    extra = {}
    if has_bias:
        g = np.arange(P * PD)
        o_of = (g // HW).reshape(P, PD)
        bU = np.einsum("kpd,kpde->kpe", conv_b[:, o_of], weights)
        bs1 = 0.1 * bU.sum(1)
        extra["biasu"] = np.broadcast_to(
            bU.reshape(1, KPE).astype(ml_dtypes.bfloat16), (TB, KPE)
        ).copy()
        extra["biasu2"] = np.broadcast_to(
            bU.transpose(0, 2, 1).reshape(1, KPE).astype(ml_dtypes.bfloat16),
            (TB, KPE),
        ).copy()
        extra["biass1"] = np.broadcast_to(
            bs1.reshape(1, KE).astype(np.float32), (TB, KE)
        ).copy()

    # -------- shard + transpose x on host: [core][bt, chunk, c, hw, b]
    xb = x.reshape(B_TOTAL, C, HW).astype(ml_dtypes.bfloat16)
    in_maps = []
    for ci in range(N_CORES):
        xs = xb[ci * B_CORE:(ci + 1) * B_CORE]            # [256, 256, 36]
        xs = xs.reshape(NT, TB, 2, 128, HW)               # bt, b, chunk, c, hw
        xT = np.ascontiguousarray(xs.transpose(0, 2, 3, 4, 1))  # bt,chunk,c,hw,b
        in_maps.append(
            {
                "xt": xT.reshape(NT, 2, 128, TB * HW),
                "cwt": cwT,
                "w2blk": w2blk,
                "w2s": w2s,
                **extra,
            }
        )

    key = has_bias
    if key not in _NC_CACHE:
        _NC_CACHE[key] = _build_nc(has_bias)
    nc = _NC_CACHE[key]

    res = run_bass_kernel_spmd(
        nc, in_maps, core_ids=list(range(N_CORES)), trace=_trace
    )
    out = np.concatenate([r["out"] for r in res.results], axis=0)
    if _trace:
        kernel._last_result = res
    return out.reshape(B_TOTAL, K, E)
